# revision 16
# baseline (speedup 1.0000x reference)
"""GPTQ-style grouped-dequant linear on 8 Trainium2 cores.

out[m,n] = sum_k A[m,k] * (q[n,k] - zeros[n,k//128]) * scales[n,k//128] + bias[n]
M=2048, K=4096, N=4096, group=128.

Sharding: 2D tensor-parallel — 2 M-halves x 4 N-quarters. Each core owns
A rows [mh*1024, ...+1024) (all K) and out-features [nq*1024, ...+1024):
A traffic halves vs pure column-parallel while per-core matmul work is
unchanged. Host does lossless layout only (transposes, uint8 repack of
4-bit values, constant one-hot selector).

Per core, per (k-group g, N-half h) unit:
  - zeros/scales rows broadcast across partitions via K=32 one-hot
    matmuls placed in distinct PE row-groups (tile_position) — emitted
    two groups ahead in bursts of four so they stream concurrently;
  - scales-broadcast converted PSUM->SBUF bf16 on the Scalar engine so
    the dequant multiply runs in DVE 2x mode;
  - dequant: DVE subtract (u8 q - zeros, bf16 out) then 2x multiply
    producing the bf16 W^T tile in [k, n] layout.
A^T tiles load as GpSimd software-DGE casting DMAs (f32 DRAM -> bf16
SBUF, no compute-engine convert, issued on the Pool queue in parallel
with Sync-queue DMAs). Output accumulates in NLEAD concurrently-open
PSUM half-chains (lead m-tiles, staggered joins with catch-up bursts)
fed as units land, then dense back-to-back chains for remaining tiles.
Bias is added at finish (DVE) from a broadcast bias row; finish DMAs
the f32 row block to DRAM. Engine queues are emitted in need-order.
"""

import numpy as np

import concourse.bass as bass
import concourse.mybir as mybir
import concourse.tile as tile
from concourse import bacc
from concourse.bass_utils import run_bass_kernel_spmd

P = 128
M, K, N = 2048, 4096, 4096
NCORES = 8
MSH, NSH = 2, 4           # M-halves x N-quarters
MC = M // MSH             # 1024 rows per core
NS = N // NSH             # 1024 out-features per core
G = K // P                # 32 groups (group_size == P == 128)
MT = MC // P              # 8 output row tiles per core
NLEAD = 4                 # concurrently-open lead half-chains

_cached = None


def _build():
    nc = bacc.Bacc("TRN2", target_bir_lowering=False, debug=False,
                   num_devices=NCORES)
    at = nc.dram_tensor("AT4", [MT, P, G, P], mybir.dt.float32,
                        kind="ExternalInput")
    qt = nc.dram_tensor("q4", [P, G, NS], mybir.dt.uint8,
                        kind="ExternalInput")
    # [z_h0 | s_h0 | z_h1 | s_h1] halves so the startup DMA is small
    zst = nc.dram_tensor("zs4", [P, 2 * NS], mybir.dt.float32,
                         kind="ExternalInput")
    bi = nc.dram_tensor("bias", [1, NS], mybir.dt.float32, kind="ExternalInput")
    oh = nc.dram_tensor("oneh", [P, G, P], mybir.dt.bfloat16,
                        kind="ExternalInput")
    out = nc.dram_tensor("out", [MC, NS], mybir.dt.float32,
                         kind="ExternalOutput")

    bf16, f32 = mybir.dt.bfloat16, mybir.dt.float32

    with tile.TileContext(nc) as tc:
        with (
            tc.tile_pool(name="const", bufs=1) as const,
            tc.tile_pool(name="qpool", bufs=1) as qpool,
            tc.tile_pool(name="sbbp", bufs=3) as sbbp,
            tc.tile_pool(name="tmp", bufs=2) as tmpp,
            tc.tile_pool(name="zbp", bufs=2, space="PSUM") as zbp,
            tc.tile_pool(name="sbp", bufs=2, space="PSUM") as sbp,
            tc.tile_pool(name="wt", bufs=1) as wtp,
            tc.tile_pool(name="apool", bufs=3) as apool,
            tc.tile_pool(name="abpool", bufs=7) as abpool,
            tc.tile_pool(name="mpsum", bufs=NLEAD, space="PSUM") as mpsum,
            tc.tile_pool(name="opool", bufs=2) as opool,
        ):
            # ---- persistent tiles ----
            ohs = const.tile([P, G, P], bf16, tag="ohs")
            zs4f = const.tile([P, 2 * NS], f32, tag="zs4f")
            z4b = const.tile([P, NS], bf16, tag="z4b")
            s4b = const.tile([P, NS], bf16, tag="s4b")
            b4f = const.tile([1, NS], f32, tag="b4f")
            b4b = const.tile([1, NS], bf16, tag="b4b")
            bias_sb = const.tile([P, NS], f32, tag="bias_sb")
            q8s = qpool.tile([P, G, NS], mybir.dt.uint8, tag="q8s")
            abt = {mt: abpool.tile([P, G, P], bf16, name="ab")
                   for mt in range(MT)}

            qr, atr, ohr = qt.ap(), at.ap(), oh.ap()

            # ---- startup: Sync queue (consts/q) in tightest-need order;
            # ---- Pool queue (A casting DMAs) runs in parallel
            nc.sync.dma_start(out=zs4f[:, 0:NS], in_=zst.ap()[:, 0:NS])
            nc.sync.dma_start(out=ohs[:, 0:8, :], in_=ohr[:, 0:8, :])
            nc.sync.dma_start(out=q8s[:, 0:4, :], in_=qr[:, 0:4, :])

            def dma_af(mt, c):
                af = apool.tile([P, G // 4, P], f32)
                nc.sync.dma_start(out=af[:],
                                  in_=atr[mt, :, c * 8:(c + 1) * 8, :])
                return af

            def cvt_ab(mt, c, af):
                nc.scalar.copy(abt[mt][:, c * 8:(c + 1) * 8, :], af[:])

            def dma_ab_cast(mt):
                # software-DGE casting DMA (f32 DRAM -> bf16 SBUF), issued
                # on the otherwise-idle Pool queue; only for dense tiles
                # whose need-time is far away
                nc.gpsimd.dma_start(out=abt[mt][:], in_=atr[mt])

            afq = {}
            afq[(0, 0)] = dma_af(0, 0)
            afq[(1, 0)] = dma_af(1, 0)
            afq[(2, 0)] = dma_af(2, 0)
            afq[(3, 0)] = dma_af(3, 0)

            # half-converted z/s (h0 now; h1 late in the h0 loop)
            nc.vector.tensor_copy(z4b[:, 0:512], zs4f[:, 0:512])
            nc.vector.tensor_copy(s4b[:, 0:512], zs4f[:, 512:1024])

            def bcast(g, h):
                """One-hot K=32 matmuls: zeros stays in PSUM (consumed by
                DVE subtract); scales gets an ACT psum->bf16 convert so
                the multiply runs in DVE 2x mode."""
                rz, rs = 2 * (g % 2), 2 * (g % 2) + 1
                zb = zbp.tile([P, 512], f32, tag="zb")
                sb = sbp.tile([P, 512], f32, tag="sb")
                nc.tensor.matmul(zb[:], ohs[32 * rz:32 * rz + 32, g, :],
                                 z4b[32 * rz:32 * rz + 32,
                                     512 * h:512 * h + 512],
                                 start=True, stop=True,
                                 tile_position=(32 * rz, 0))
                nc.tensor.matmul(sb[:], ohs[32 * rs:32 * rs + 32, g, :],
                                 s4b[32 * rs:32 * rs + 32,
                                     512 * h:512 * h + 512],
                                 start=True, stop=True,
                                 tile_position=(32 * rs, 0))
                return zb, sb

            def cvt_sbb(sb):
                sbb = sbbp.tile([P, 512], bf16)
                nc.scalar.copy(sbb[:], sb[:])
                return sbb

            wts = {}

            def dequant(g, h, zb, sbb):
                tmp = tmpp.tile([P, 512], bf16)
                nc.vector.tensor_tensor(tmp[:], q8s[:, g, 512 * h:512 * h + 512],
                                        zb[:], mybir.AluOpType.subtract)
                wt = wtp.tile([P, 512], bf16, tag=f"wt{g}_{h}")
                nc.vector.tensor_tensor(wt[:], tmp[:], sbb[:],
                                        mybir.AluOpType.mult)
                wts[(g, h)] = wt

            def finish(mt, h, ps):
                ob = opool.tile([P, 512], f32)
                nc.vector.tensor_tensor(ob[:], ps[:],
                                        bias_sb[:, 512 * h:512 * h + 512],
                                        mybir.AluOpType.add)
                nc.sync.dma_start(
                    out=out.ap()[mt * P:(mt + 1) * P, 512 * h:512 * h + 512],
                    in_=ob[:])

            # scheduled extra work during the h0 loop, keyed by unit index
            def h0_extras(g):
                # lead A chunks: DMA two units before the ACT convert,
                # convert two units before first use at group 8c
                if g % 8 == 2 and g < 24:
                    c = g // 8 + 1
                    for mt in range(NLEAD):
                        afq[(mt, c)] = dma_af(mt, c)
                if g % 8 == 4 and g < 24:
                    c = g // 8 + 1
                    for mt in range(NLEAD):
                        cvt_ab(mt, c, afq.pop((mt, c)))
                if g == 2:
                    nc.vector.tensor_copy(b4b[:], b4f[:])
                if g in (4, 5):
                    # bias broadcast: K=1 ones-row x bias row -> psum,
                    # ACT-copy to SBUF f32; consumed by every finish
                    h = g - 4
                    pb = zbp.tile([P, 512], f32, tag="zb")
                    nc.tensor.matmul(pb[:], ohs[0:1, 0, :],
                                     b4b[0:1, 512 * h:512 * h + 512],
                                     start=True, stop=True,
                                     tile_position=(0, 0))
                    nc.scalar.copy(bias_sb[:, 512 * h:512 * h + 512],
                                   pb[:])
                if g == 26:
                    nc.vector.tensor_copy(z4b[:, 512:1024],
                                          zs4f[:, NS:NS + 512])
                if g == 28:
                    nc.vector.tensor_copy(s4b[:, 512:1024],
                                          zs4f[:, NS + 512:NS + 1024])
                if g == 1:
                    nc.sync.dma_start(out=b4f[:], in_=bi.ap()[:])
                if g == 20:
                    nc.sync.dma_start(out=zs4f[:, NS:2 * NS],
                                      in_=zst.ap()[:, NS:2 * NS])
                # q chunks (2 groups each): chunk k needed at group 2k
                if g % 2 == 1 and g < 29:
                    k = (g + 3) // 2
                    nc.sync.dma_start(out=q8s[:, 2 * k:2 * k + 2, :],
                                      in_=qr[:, 2 * k:2 * k + 2, :])
                # one-hot chunks
                if g % 8 == 3 and g < 24:
                    c = g // 8 + 1
                    nc.sync.dma_start(out=ohs[:, 8 * c:8 * c + 8, :],
                                      in_=ohr[:, 8 * c:8 * c + 8, :])

            # dense-tail A tiles: one casting DMA each, spread through h1
            def h1_extras(g):
                if g % 8 == 1:
                    dma_ab_cast(NLEAD + g // 8)

            def unit_phase(h, join_at, extras, fins):
                ps = {}
                pend = {0: bcast(0, h), 1: bcast(1, h)}

                def lead_mm(mt, g, first):
                    nc.tensor.matmul(ps[mt][:], abt[mt][:, g, :],
                                     wts[(g, h)][:], start=first,
                                     stop=(g == G - 1))

                for g2 in range(0, G, 2):
                    for g in (g2, g2 + 1):
                        zb, sb = pend.pop(g)
                        sbb = cvt_sbb(sb)
                        dequant(g, h, zb, sbb)
                        extras(g)
                        if g < len(fins):  # previous phase's finishes
                            fins[g]()
                    for gn in (g2 + 2, g2 + 3):  # burst of 4 bcast matmuls
                        if gn < G:
                            pend[gn] = bcast(gn, h)
                    for g in (g2, g2 + 1):
                        for mt in range(NLEAD):
                            if join_at[mt] == g:
                                ps[mt] = mpsum.tile([P, 512], f32, name="ps")
                                for gc in range(g + 1):  # catch-up burst
                                    lead_mm(mt, gc, gc == 0)
                            elif join_at[mt] < g:
                                lead_mm(mt, g, False)
                return [lambda mt=mt, p=ps[mt]: finish(mt, h, p)
                        for mt in range(NLEAD)]

            # first lead A converts in ACT need-order (after first sbb cvt)
            def h0_extras_first(g):
                if g == 0:
                    cvt_ab(0, 0, afq.pop((0, 0)))
                    cvt_ab(1, 0, afq.pop((1, 0)))
                if g == 1:
                    cvt_ab(2, 0, afq.pop((2, 0)))
                    cvt_ab(3, 0, afq.pop((3, 0)))
                h0_extras(g)

            f0 = unit_phase(0, {0: 0, 1: 0, 2: 2, 3: 2}, h0_extras_first, [])
            f1 = unit_phase(1, {0: 0, 1: 0, 2: 2, 3: 2}, h1_extras, f0)
            for f in f1:
                f()

            # dense tail: remaining m-tiles, back-to-back matmul chains
            for mt in range(NLEAD, MT):
                for h in range(2):
                    ps = mpsum.tile([P, 512], f32, name="ps")
                    for g in range(G):
                        nc.tensor.matmul(ps[:], abt[mt][:, g, :],
                                         wts[(g, h)][:], start=(g == 0),
                                         stop=(g == G - 1))
                    finish(mt, h, ps)

    nc.compile()
    return nc


def _prep_inputs(A, qweight, scales, zeros, bias):
    import ml_dtypes
    # one-hot selector: oneh[32i + j, g, m] = (j == g), constant
    base = np.zeros((32, G, P), dtype=ml_dtypes.bfloat16)
    for j in range(32):
        base[j, j, :] = 1.0
    oneh = np.ascontiguousarray(np.tile(base, (4, 1, 1)))
    # AT4[mt, p, g, j] = A[mh*1024 + mt*128 + j, g*128 + p]
    at4 = [np.ascontiguousarray(
        A[mh * MC:(mh + 1) * MC].reshape(MT, P, G, P).transpose(0, 3, 2, 1))
        for mh in range(MSH)]
    in_maps = []
    for c in range(NCORES):
        mh, nq = c // NSH, c % NSH
        r = slice(nq * NS, (nq + 1) * NS)
        # q4[p, g, n] = q[n, g*128+p]
        q4 = np.ascontiguousarray(
            qweight[r].astype(np.uint8).T.reshape(G, P, NS).transpose(1, 0, 2))
        # 4 stacked copies of [G, NS_half] blocks: [z_h0 | s_h0 | z_h1 | s_h1]
        zt = np.tile(zeros[r].T, (4, 1))
        st = np.tile(scales[r].T, (4, 1))
        zs4 = np.ascontiguousarray(np.concatenate(
            [zt[:, 0:512], st[:, 0:512], zt[:, 512:1024], st[:, 512:1024]],
            axis=1))
        in_maps.append({
            "AT4": at4[mh],
            "q4": q4,
            "zs4": zs4,
            "bias": np.ascontiguousarray(bias[r]).reshape(1, NS),
            "oneh": oneh,
        })
    return in_maps


def run(inputs, **spmd_kwargs):
    global _cached
    if _cached is None:
        _cached = _build()
    in_maps = _prep_inputs(**inputs)
    res = run_bass_kernel_spmd(_cached, in_maps, list(range(NCORES)),
                               **spmd_kwargs)
    rows = [np.concatenate([res.results[mh * NSH + nq]["out"]
                            for nq in range(NSH)], axis=1)
            for mh in range(MSH)]
    return np.concatenate(rows, axis=0), res


def kernel(**inputs):
    return run(inputs)[0]
